# revision 16
# baseline (speedup 1.0000x reference)
"""Gaussian falloff vortex-velocity kernel for Trainium2 (8 NeuronCores).

Math: out[b,h,w,:] = sum_n tau_n * exp(-r2/sig_n^2) / sqrt(r2) * (d2, -d1)
with d1 = py - y_n, d2 = px - x_n, r2 = d1^2 + d2^2.

Device algorithm (per core, H sharded 8 ways):
  1. PE computes t2' = a_n*(r2 + eps_n) for 128 particles x 512 points per
     matmul, where a_n = 2/sig_n^2, via a K=31 contraction of triple-bf16-split
     terms: a*py^2 - 2a*y*py + a*y^2 + a*px^2 - 2a*x*px + a*x^2 + a*eps.
     Rows are ordered so partial sums telescope near zero for close pairs,
     keeping fp32 accumulation error ~1e-6 in r2 units.
  2. ACT: lt = Ln(t2')                 (PSUM -> SBUF, fp32)
  3. DVE: w  = -t2' - lt               (one scalar_tensor_tensor, fp32)
  4. ACT: g  = Exp(0.5*w)              (-> bf16)  [= exp(-t2'/2)/sqrt(t2')]
  5. PE: S_r = sum_n w_rn * g_n  for r in {0,1,2} with hi/lo-split bf16
     weights {tau*q, tau*x*q, tau*y*q}, q = exp(a*eps/2)*sqrt(a).
  6. DVE: u = px*S0 - S1, v = S2 - py*S0 (after a DRAM relayout round-trip).
Ln and Exp share one ACT table set (natural_log_exp_and_others).
"""

import sys

import numpy as np

B, H, W, N = 2, 256, 256, 512
NCORES = 8
HPC = H // NCORES          # 32 rows per core
PPB = HPC * W              # 8192 points per batch per core
NT = PPB // 512            # 16 point-tiles of 512 per batch
NK = N // 128              # 4 particle blocks
KROWS = 31
EPS0, EPS1 = 2e-6, 1.5e-6

_cache = {}


def _bass_modules():
    if "/opt/trn_rl_repo" not in sys.path:
        sys.path.insert(0, "/opt/trn_rl_repo")
    import concourse.bass as bass
    import concourse.mybir as mybir
    import concourse.tile as tile
    from concourse import bacc
    from concourse.bass_utils import run_bass_kernel_spmd

    return bass, mybir, tile, run_bass_kernel_spmd, bacc


def _pin_act_table_set():
    """Make the table-load pass satisfy Ln/Exp only from the combined set so
    alternating Ln/Exp instructions never thrash ACT table loads."""
    import concourse.bacc as bacc_mod
    import concourse.mybir as mybir

    if getattr(bacc_mod, "_act_tables_pinned", False):
        return
    orig = bacc_mod.get_activation_tables
    ln_exp = {mybir.ActivationFunctionType.Ln, mybir.ActivationFunctionType.Exp}

    def patched(arch):
        tables = orig(arch)
        keep = "natural_log_exp_and_others"
        if keep not in tables:
            return tables
        return {
            name: (funcs if name == keep else (funcs - ln_exp))
            for name, funcs in tables.items()
        }

    bacc_mod.get_activation_tables = patched
    bacc_mod._act_tables_pinned = True


def _build_nc():
    bass, mybir, tile, _, bacc = _bass_modules()
    _pin_act_table_set()
    f32 = mybir.dt.float32
    bf16 = mybir.dt.bfloat16
    AF = mybir.ActivationFunctionType
    ALU = mybir.AluOpType

    nc = bacc.Bacc(None)
    rhs_d = nc.declare_dram_parameter("rhs", [B, 8, KROWS, PPB // 8], bf16, isOutput=False)
    lhst_d = nc.declare_dram_parameter("lhst", [B, KROWS, N], bf16, isOutput=False)
    wm_d = nc.declare_dram_parameter("wmat", [128, B * NK * 6], bf16, isOutput=False)
    ptsf_d = nc.declare_dram_parameter("ptsf", [B, 2, 128, PPB // 128], f32, isOutput=False)
    out_d = nc.declare_dram_parameter("out", [B, 2, 128, PPB // 128], f32, isOutput=True)

    with tile.TileContext(nc) as tc:
        with (
            tc.tile_pool(name="const", bufs=1) as cpool,
            tc.tile_pool(name="lts", bufs=3) as ltpool,
            tc.tile_pool(name="wg", bufs=2) as wgpool,
            tc.tile_pool(name="stg", bufs=2) as stgpool,
            tc.tile_pool(name="fin", bufs=2) as fin,
            tc.tile_pool(name="r2p", bufs=3, space=bass.MemorySpace.PSUM) as r2pool,
            tc.tile_pool(name="sap", bufs=2, space=bass.MemorySpace.PSUM) as spool,
            tc.tile_pool(name="dscratch", bufs=1, space="DRAM") as dpool,
        ):
            rhs_sb, lhs_sb = [], []
            for b in range(B):
                t = cpool.tile([KROWS, PPB], bf16, tag=f"rhs{b}")
                rhs_sb.append(t)
                t2 = cpool.tile([KROWS, N], bf16, tag=f"lhs{b}")
                lhs_sb.append(t2)
            # first compute chunk unblocks ASAP, rest follow
            nc.sync.dma_start(rhs_sb[0][:, 0 : PPB // 8], rhs_d[0, 0])
            nc.sync.dma_start(lhs_sb[0][:], lhst_d[0])
            wm = cpool.tile([128, B * NK * 6], bf16, tag="wm")
            nc.sync.dma_start(wm[:], wm_d[:])
            for c in range(1, 8):
                cs = slice(c * (PPB // 8), (c + 1) * (PPB // 8))
                nc.sync.dma_start(rhs_sb[0][:, cs], rhs_d[0, c])
            nc.sync.dma_start(lhs_sb[1][:], lhst_d[1])
            for c in range(8):
                cs = slice(c * (PPB // 8), (c + 1) * (PPB // 8))
                nc.sync.dma_start(rhs_sb[1][:, cs], rhs_d[1, c])
            ptq = {}
            for b in range(B):
                for q in range(4):
                    pq = cpool.tile([32, 128], f32, tag=f"ptq{b}{q}", name=f"ptq{b}{q}")
                    nc.sync.dma_start(
                        pq[:, 0:64], ptsf_d[b, 0, q * 32 : (q + 1) * 32]
                    )
                    nc.sync.dma_start(
                        pq[:, 64:128], ptsf_d[b, 1, q * 32 : (q + 1) * 32]
                    )
                    ptq[(b, q)] = pq
            scratch = dpool.tile([B, 4, 6, PPB // 4], f32)
            srs = scratch[:].rearrange("b q six (p f) -> b q p six f", p=32)

            NTP = NT // 2  # tile-pair groups per batch
            wt_t, g_t, sacc_t, stage_t = {}, {}, {}, {}

            def stage_a(gi, tt):  # r2 matmuls + Ln + stt for group gi, half tt
                b, TP = divmod(gi, NTP)
                if tt == 0:
                    wt_t[gi] = wgpool.tile(
                        [128, 4096], f32, tag="wt", name=f"wt{gi}"
                    )
                wt = wt_t[gi]
                if TP == 0 and tt == 0:
                    stage_t[b] = stgpool.tile(
                        [6, PPB], f32, tag="sstage", name=f"sstage{b}"
                    )
                if True:
                    T = TP * 2 + tt
                    sl = slice(T * 512, (T + 1) * 512)
                    for p in range(2):
                        r2t = r2pool.tile([128, 1024], f32, tag="r2")
                        for hh in range(2):
                            k = 2 * p + hh
                            nc.tensor.matmul(
                                r2t[:, hh * 512 : (hh + 1) * 512],
                                lhs_sb[b][:, k * 128 : (k + 1) * 128],
                                rhs_sb[b][:, sl],
                                start=True,
                                stop=True,
                            )
                        lt = ltpool.tile([128, 1024], f32, tag="lt")
                        nc.scalar.activation(lt[:], r2t[:], AF.Ln)
                        nc.vector.scalar_tensor_tensor(
                            wt[:, (tt * 2 + p) * 1024 : (tt * 2 + p + 1) * 1024],
                            r2t[:],
                            -1.0,
                            lt[:],
                            ALU.mult,
                            ALU.subtract,
                        )

            def stage_b(gi):  # Exp for group gi
                g = wgpool.tile([128, 4096], bf16, tag="g", bufs=3)
                g_t[gi] = g
                nc.scalar.activation(g[:], wt_t.pop(gi)[:], AF.Exp, scale=0.5)

            def stage_c(gi, tt):  # S-matmuls + staging copy for group gi, half tt
                b, TP = divmod(gi, NTP)
                g = g_t[gi] if tt == 0 else g_t.pop(gi)
                sstage = stage_t[b]
                T = TP * 2 + tt
                sl = slice(T * 512, (T + 1) * 512)
                sacc = spool.tile([6, 512], f32, tag="sacc")
                for k in range(NK):
                    c6 = (b * NK + k) * 6
                    gk = g[:, (tt * NK + k) * 512 : (tt * NK + k + 1) * 512]
                    nc.tensor.matmul(
                        sacc[:], wm[:, c6 : c6 + 6], gk,
                        start=(k == 0), stop=(k == NK - 1),
                    )
                nc.vector.tensor_copy(sstage[:, sl], sacc[:])
                if tt == 1 and TP % 2 == 1:
                    finish_quarter(b, TP // 2)

            QW = PPB // 4  # points per quarter

            def finish_quarter(b, q):
                sstage = stage_t[b]
                qs = slice(q * QW, (q + 1) * QW)
                nc.sync.dma_start(scratch[b, q], sstage[:, qs])
                sq = fin.tile([32, 384], f32, tag="sq")
                nc.sync.dma_start(
                    sq[:].rearrange("p (six f) -> p six f", six=6), srs[b, q]
                )
                sh = [sq[:, rr * 64 : (rr + 1) * 64] for rr in range(6)]
                pyf = ptq[(b, q)][:, 0:64]
                pxf = ptq[(b, q)][:, 64:128]
                s0 = fin.tile([32, 64], f32, tag="s0t")
                nc.vector.tensor_add(s0[:], sh[0], sh[3])
                s1 = fin.tile([32, 64], f32, tag="s1t")
                nc.vector.tensor_add(s1[:], sh[1], sh[4])
                s2 = fin.tile([32, 64], f32, tag="s2t")
                nc.vector.tensor_add(s2[:], sh[2], sh[5])
                tu = fin.tile([32, 64], f32, tag="tu")
                nc.vector.tensor_mul(tu[:], pxf, s0[:])
                u = fin.tile([32, 64], f32, tag="u")
                nc.vector.tensor_sub(u[:], tu[:], s1[:])
                tv = fin.tile([32, 64], f32, tag="tv")
                nc.vector.tensor_mul(tv[:], pyf, s0[:])
                v = fin.tile([32, 64], f32, tag="v")
                nc.vector.tensor_sub(v[:], s2[:], tv[:])
                nc.sync.dma_start(out_d[b, 0, q * 32 : (q + 1) * 32], u[:])
                nc.sync.dma_start(out_d[b, 1, q * 32 : (q + 1) * 32], v[:])

            NG = B * NTP
            STEP = 0.008  # ms of logical time per pipeline iteration
            for gi in range(NG + 2):
                t_it = STEP * gi
                if gi < NG:
                    with tc.tile_wait_until(t_it):
                        stage_a(gi, 0)
                if 2 <= gi <= NG + 1:
                    with tc.tile_wait_until(t_it + 0.002):
                        stage_c(gi - 2, 0)
                if 1 <= gi <= NG:
                    with tc.tile_wait_until(t_it + 0.003):
                        stage_b(gi - 1)
                if gi < NG:
                    with tc.tile_wait_until(t_it + 0.004):
                        stage_a(gi, 1)
                if 2 <= gi <= NG + 1:
                    with tc.tile_wait_until(t_it + 0.007):
                        stage_c(gi - 2, 1)
    nc.compile()
    return nc


def _split3(a, bf):
    h = a.astype(bf)
    m = (a - h.astype(np.float64)).astype(bf)
    l = (a - h.astype(np.float64) - m.astype(np.float64)).astype(bf)
    return h, m, l


def _prep_inputs(vortex_feature, points):
    import ml_dtypes

    bf = ml_dtypes.bfloat16
    vf = np.asarray(vortex_feature, dtype=np.float64)
    pts_full = np.asarray(points, dtype=np.float64)
    y, x, tau = vf[:, :, 0], vf[:, :, 1], vf[:, :, 2]
    sig2 = vf[:, :, 3] ** 2
    a_n = 2.0 / sig2
    eps_n = EPS0 + EPS1 * (y * y + x * x)

    # lhsT rows [B, KROWS, N]: triple-split entries; order must match rhs rows.
    lhst = np.zeros((B, KROWS, N), dtype=bf)
    for b in range(B):
        A3 = _split3(a_n[b], bf)
        CY3 = _split3(-2.0 * a_n[b] * y[b], bf)
        CX3 = _split3(-2.0 * a_n[b] * x[b], bf)
        AYY3 = _split3(a_n[b] * y[b] * y[b], bf)
        AXX3 = _split3(a_n[b] * x[b] * x[b], bf)
        aeps = (a_n[b] * eps_n[b]).astype(bf)
        rows = []
        for (uh, um, ul) in (A3, CY3):
            rows += [uh, uh, um, uh, ul, um]
        rows += list(AYY3)
        for (uh, um, ul) in (A3, CX3):
            rows += [uh, uh, um, uh, ul, um]
        rows += list(AXX3)
        rows.append(aeps)
        lhst[b] = np.stack(rows, 0)

    # weights with eps correction, hi/lo split side by side: [128, B*NK*6]
    q = np.exp(0.5 * a_n * eps_n) * np.sqrt(a_n)
    wfull = np.stack([tau * q, tau * x * q, tau * y * q], axis=-1)  # [B, N, 3]
    whd = wfull.astype(bf)
    wld = (wfull - whd.astype(np.float64)).astype(bf)
    w6 = np.concatenate([whd, wld], axis=-1)  # [B, N, 6]
    wm = np.ascontiguousarray(
        w6.reshape(B, NK, 128, 6).transpose(2, 0, 1, 3).reshape(128, B * NK * 6)
    )

    in_maps = []
    for i in range(NCORES):
        slp = pts_full[:, i * HPC : (i + 1) * HPC].reshape(B, PPB, 2)
        pts = np.ascontiguousarray(slp.transpose(0, 2, 1))  # [B, 2, PPB]
        ptsf = np.ascontiguousarray(
            pts.reshape(B, 2, 128, PPB // 128), dtype=np.float32
        )
        rhs = np.zeros((B, KROWS, PPB), dtype=bf)
        for b in range(B):
            py, px = pts[b, 0], pts[b, 1]
            PYY3 = _split3(py * py, bf)
            PY3 = _split3(py, bf)
            PXX3 = _split3(px * px, bf)
            PX3 = _split3(px, bf)
            ones = np.ones(PPB, dtype=bf)
            rows = []
            for (wh_, wm_, wl_) in (PYY3, PY3):
                rows += [wh_, wm_, wh_, wl_, wh_, wm_]
            rows += [ones] * 3
            for (wh_, wm_, wl_) in (PXX3, PX3):
                rows += [wh_, wm_, wh_, wl_, wh_, wm_]
            rows += [ones] * 3
            rows.append(ones)
            rhs[b] = np.stack(rows, 0)
        rhs8 = np.ascontiguousarray(
            rhs.reshape(B, KROWS, 8, PPB // 8).transpose(0, 2, 1, 3)
        )
        in_maps.append({"rhs": rhs8, "lhst": lhst, "wmat": wm, "ptsf": ptsf})
    return in_maps


def _assemble(results):
    out = np.zeros((B, H, W, 2), dtype=np.float32)
    for i in range(NCORES):
        o = np.asarray(results[i]["out"])  # [B, 2, 128, PPB//128]
        o = o.reshape(B, 2, PPB).transpose(0, 2, 1).reshape(B, HPC, W, 2)
        out[:, i * HPC : (i + 1) * HPC] = o
    return out


def _run(vortex_feature, points, trace=False):
    _, _, _, run_bass_kernel_spmd, _b = _bass_modules()
    if "nc" not in _cache:
        _cache["nc"] = _build_nc()
    in_maps = _prep_inputs(vortex_feature, points)
    res = run_bass_kernel_spmd(
        _cache["nc"], in_maps, list(range(NCORES)), trace=trace
    )
    return _assemble(res.results), res


def kernel(vortex_feature, points):
    out, _ = _run(vortex_feature, points, trace=False)
    return out
